# revision 27
# baseline (speedup 1.0000x reference)
"""Bass/Trainium2 kernel for nn_KernelAMController (retrieval_knn).

Math: out(b,:) = -sum_g w_eff(b,g)*adj[tb(b),g,:] / (sum_g w_eff(b,g) + eps)
with w_eff(b,g) = exp(-2*||x_b - p_g||^2) * (counts[tb(b),g] > 0).

Strategy: the Gaussian (bandwidth 0.5) support radius is ~1.6, so after the
host sorts queries by x-coordinate each 512-sample group only interacts
with a narrow x-band of the grid. The grid is gathered host-side at
64-point granularity into per-group operand blocks of 128 points; a static
per-group-index slot profile (computed from the data, groups reordered
within each core to fit it) keeps the compiled program identical on all
cores while padding as little dummy work as possible.

Per group:
  mm1: W^T = exp(Pa^T @ Xa) over the selected blocks, K=15 split-bf16
       augmented matmul, 3 blocks CONCURRENT in the PE via row tiling
       (partition bases 0/32/64/96), exp in N=512*m ACTIVATE batches.
  mm2: Y^T(m,b) += Ct(g,m)*W^T(g,b); even slots -> PSUM 0:64, odd ->
       64:128 (col tiling, concurrent).
  Host-precomputed one-hot bin mask (o3full) * Y^T, reduced over bins by
  a +/-1 block matmul (output negation folded in). The [3,BG] result
  (num_x, num_y, den) goes back to the host, which divides/unsorts.
Inputs stream in per-group tiles spread across four engine DMA queues so
group 0's operands land ~100KB into the transfer instead of 3MB.
"""
import numpy as np
import ml_dtypes

import concourse.bass as bass
import concourse.tile as tile
from concourse import mybir, bacc
from concourse.bass_utils import run_bass_kernel_spmd

F32 = mybir.dt.float32
BF16 = mybir.dt.bfloat16
BF16_NP = ml_dtypes.bfloat16

B = 32768
G = 2500
GP = 2560          # padded grid (20 chunks of 128)
NGRAN = 40         # 64-point granules
NBINS = 20
NCORES = 8
BC = B // NCORES   # 4096 samples per core
NGRP = 8           # groups per core
BG = BC // NGRP    # 512 samples per group
MAXBLK = 6         # hard cap on 128-point blocks per group
RADIUS = 1.6       # x-distance truncation for granule selection
EPS = 1e-10

_CACHE = {}


def _batches(nblk):
    """Split a block count into mm1 batch sizes (3s then a 2/remainder)."""
    out = []
    while nblk >= 3:
        if nblk == 4:
            out += [2, 2]
            nblk = 0
            break
        out.append(3)
        nblk -= 3
    if nblk:
        out.append(nblk)
    return out


def _build_nc(profile):
    nblks = list(profile)
    tot_slots = sum(nblks)
    tot_batches = sum(len(_batches(n)) for n in nblks)

    nc = bacc.Bacc("TRN2", target_bir_lowering=False)
    # per-group widths of the grid-side gather buffer (pa batches + ct)
    gws = [len(_batches(n)) * 128 + n * 64 for n in nblks]
    gin_d = nc.dram_tensor("gin", [128, sum(gws)], BF16,
                           kind="ExternalInput")
    xo_d = nc.dram_tensor("xo", [128, NGRP * 2 * BG], BF16,
                          kind="ExternalInput")
    bn_d = nc.dram_tensor("bn128", [128, 3], BF16, kind="ExternalInput")
    o_d = nc.dram_tensor("o", [NGRP, 3, BG], F32, kind="ExternalOutput")

    with tile.TileContext(nc) as tc:
        with (
            tc.tile_pool(name="consts", bufs=1) as consts,
            tc.tile_pool(name="wt", bufs=4) as wtp,
            tc.tile_pool(name="r3", bufs=2) as r3p,
            tc.tile_pool(name="os", bufs=2) as osp,
            tc.tile_pool(name="pw", bufs=2, space="PSUM") as pwp,
            tc.tile_pool(name="py", bufs=1, space="PSUM") as pyp,
            tc.tile_pool(name="pr", bufs=1, space="PSUM") as prp,
        ):
            # warm the exp table load under the input DMAs
            dm = consts.tile([1, 1], F32)
            nc.vector.memset(dm[:], 0.0)
            dm2 = consts.tile([1, 1], F32)
            nc.scalar.activation(dm2[:], dm[:],
                                 mybir.ActivationFunctionType.Exp)

            bn_sb = consts.tile([128, 3], BF16)
            nc.gpsimd.dma_start(out=bn_sb[:], in_=bn_d[:])

            # Input tiles on two queues, never ScalarE's (exp must not sit
            # behind DMA issues in its FIFO). Engine waits coarsen to "all
            # prior DMAs on that ring", so the sync ring carries ONLY
            # group 0's two inputs — the first matmul gates on ~0.4MB
            # instead of the whole 3MB transfer.
            gin_t, xo_t = [], []
            goff = 0
            for g in range(NGRP):
                q = nc.sync if g == 0 else nc.gpsimd
                gin = consts.tile([128, gws[g]], BF16, tag=f"gin{g}")
                q.dma_start(out=gin[:], in_=gin_d[:, goff:goff + gws[g]])
                xo = consts.tile([128, 2 * BG], BF16, tag=f"xo{g}")
                q.dma_start(
                    out=xo[:], in_=xo_d[:, g * 2 * BG:(g + 1) * 2 * BG])
                gin_t.append(gin)
                xo_t.append(xo)
                goff += gws[g]

            # All profile entries are clamped to >=4 blocks so every group
            # has exactly two mm1 batches. Emission interleaves one group
            # ahead at BATCH granularity:
            #   M1E(0,0) M1E(0,1) | MM2(g,0) M1E(g+1,0) MM2(g,1) TAIL(g)
            #   M1E(g+1,1) | ... so exp(g+1,0) runs while the PE works on
            # group g's tail instead of idling ~1.8us per group.
            def m1e(g, bi, first_slot, m):
                pw = pwp.tile([128, 3, BG], F32)
                for k in range(m):
                    rb = 32 * ((first_slot + k) % 4)
                    nc.tensor.matmul(
                        pw[:, k, :],
                        lhsT=gin_t[g][rb:rb + 15, bi * 128:(bi + 1) * 128],
                        rhs=xo_t[g][rb:rb + 15, 0:BG],
                        start=True, stop=True, tile_position=(rb, 0))
                wt = wtp.tile([128, 3, BG], BF16)
                nc.scalar.activation(wt[:, 0:m, :], pw[:, 0:m, :],
                                     mybir.ActivationFunctionType.Exp)
                return wt

            def mm2_batch(g, py, wt_, ps, pm):
                nblk = nblks[g]
                ctoff = len(_batches(nblk)) * 128
                for k in range(pm):
                    sl = ps + k
                    out = py[0:64] if sl % 2 == 0 else py[64:128]
                    nc.tensor.matmul(
                        out,
                        lhsT=gin_t[g][:, ctoff + sl * 64:
                                      ctoff + (sl + 1) * 64],
                        rhs=wt_[:, k, :], start=(sl < 2),
                        stop=(sl >= nblk - 2), skip_group_check=True)

            def tail(g, py):
                # bin-select then reduce over bins (negation in bn128);
                # rows 60:64 / 124:128 of o3full are host-zeroed and the
                # matching py rows are exact zeros (ct pad columns)
                r3 = r3p.tile([128, BG], BF16)
                nc.vector.tensor_mul(r3[:], py[:], xo_t[g][:, BG:2 * BG])
                pr = prp.tile([3, BG], F32)
                nc.tensor.matmul(pr[:], lhsT=bn_sb[:], rhs=r3[:],
                                 start=True, stop=True)
                osb = osp.tile([3, BG], F32)
                nc.vector.tensor_copy(osb[:], pr[:])
                nc.sync.dma_start(out=o_d[g], in_=osb[:])

            bat = [_batches(n) for n in nblks]
            slot0 = [[sum(bs[:i]) for i in range(len(bs))] for bs in bat]
            wts = {}
            for bi in range(2):
                wts[(0, bi)] = m1e(0, bi, slot0[0][bi], bat[0][bi])
            for g in range(NGRP):
                py = pyp.tile([128, BG], F32)
                mm2_batch(g, py, wts.pop((g, 0)), slot0[g][0], bat[g][0])
                if g + 1 < NGRP:
                    wts[(g + 1, 0)] = m1e(g + 1, 0, slot0[g + 1][0],
                                          bat[g + 1][0])
                mm2_batch(g, py, wts.pop((g, 1)), slot0[g][1], bat[g][1])
                tail(g, py)
                if g + 1 < NGRP:
                    wts[(g + 1, 1)] = m1e(g + 1, 1, slot0[g + 1][1],
                                          bat[g + 1][1])
    nc.compile()
    return nc


def _host_prep(t, x, grid_points, grid_adjoints, t_edges, grid_counts):
    t = np.asarray(t, np.float32).reshape(B)
    x = np.asarray(x, np.float32)
    gp = np.asarray(grid_points, np.float32)
    adj = np.asarray(grid_adjoints, np.float32)
    te = np.asarray(t_edges, np.float32)
    cnt = np.asarray(grid_counts)

    order = np.argsort(x[:, 0], kind="stable")

    # granule x-extents (points are x-major: idx = ix*50 + iy)
    gx = gp[:, 0]
    gran_xmin = np.array([gx[64 * u] for u in range(NGRAN)], np.float32)
    gran_xmax = np.array([gx[min(64 * u + 63, G - 1)]
                          for u in range(NGRAN)], np.float32)

    # grid operands, bf16 hi/lo split; granule NGRAN is an all-dummy pad
    # (exponent -1e30 -> w=0, adjoints 0)
    p5 = np.zeros((5, GP + 64), np.float32)
    p5[0, :G] = 4.0 * gp[:, 0]
    p5[1, :G] = 4.0 * gp[:, 1]
    p5[2, :G] = -2.0
    p5[3, :G] = -2.0
    p5[4, :G] = -2.0 * (gp[:, 0] ** 2 + gp[:, 1] ** 2)
    p5[4, G:] = -1e30
    ph = p5.astype(BF16_NP)
    pl = (p5 - ph.astype(np.float32)).astype(BF16_NP)
    pa15 = np.concatenate([ph, ph, pl], axis=0)        # (15, GP+64) bf16

    mask = (cnt > 0).astype(np.float32)                # (20, G)
    ct = np.zeros((GP + 64, 64), np.float32)
    ct[:G, 0:20] = (mask * adj[:, :, 0]).T
    ct[:G, 20:40] = (mask * adj[:, :, 1]).T
    ct[:G, 40:60] = mask.T
    ct64 = ct.reshape(NGRAN + 1, 64, 64).astype(BF16_NP)

    # per-group granule selection on the x-sorted data
    xs0 = x[order, 0]
    sels, nblks = [], []
    for gg in range(B // BG):
        seg = xs0[gg * BG:(gg + 1) * BG]
        a, b = seg.min(), seg.max()
        d = np.maximum(np.maximum(gran_xmin - b, a - gran_xmax), 0.0)
        near = np.argsort(d, kind="stable")[:2 * MAXBLK]
        sel = np.sort(near[d[near] <= RADIUS])
        sels.append(sel)
        nblks.append((len(sel) + 1) // 2)

    # reorder groups within each core (descending block count) and derive
    # the static per-index profile shared by all cores
    nblks = np.array(nblks).reshape(NCORES, NGRP)
    perm = np.argsort(-nblks, axis=1, kind="stable")   # (NCORES, NGRP)
    profile = tuple(max(int(v), 4) for v in
                    np.max(np.take_along_axis(nblks, perm, axis=1), axis=0))
    order = order.reshape(NCORES, NGRP, BG)
    order = np.take_along_axis(order, perm[:, :, None], axis=1).reshape(-1)

    ts = t[order]
    xs = x[order]

    # x augmentation rows: [xh(2), sqh(2), 1, xl(2), sql(2), 0, xh(2),
    # sqh(2), 1] matching pa15 = [ph, ph, pl]; replicated at partition
    # bases 0/32/64/96 for PE row tiling
    xT = xs.T
    sq = xT * xT
    xh = xT.astype(BF16_NP)
    xl = (xT - xh.astype(np.float32)).astype(BF16_NP)
    sqh = sq.astype(BF16_NP)
    sql = (sq - sqh.astype(np.float32)).astype(BF16_NP)
    xaug = np.zeros((15, B), BF16_NP)
    for base in (0, 5, 10):
        xaug[base + 0:base + 2] = xl if base == 5 else xh
        xaug[base + 2:base + 4] = sql if base == 5 else sqh
        xaug[base + 4] = BF16_NP(0.0 if base == 5 else 1.0)
    xa4 = np.zeros((128, B), BF16_NP)
    for rb in (0, 32, 64, 96):
        xa4[rb:rb + 15] = xaug

    # host-computed one-hot bin mask, d-major layout matching py rows
    tb = np.clip(np.searchsorted(te[1:NBINS], ts, side="left"),
                 0, NBINS - 1)
    oh = np.zeros((NBINS, B), BF16_NP)
    oh[tb, np.arange(B)] = BF16_NP(1.0)
    o3full = np.zeros((128, B), BF16_NP)
    for half in (0, 64):
        for dd in range(3):
            o3full[half + dd * 20:half + (dd + 1) * 20] = oh

    bn = np.zeros((128, 3), np.float32)
    for dd in range(3):
        v = 1.0 if dd == 2 else -1.0
        bn[dd * 20:(dd + 1) * 20, dd] = v
        bn[64 + dd * 20:64 + (dd + 1) * 20, dd] = v
    bn = bn.astype(BF16_NP)

    # per-group widths of the grid-side gather buffer (pa batches + ct)
    gws = [len(_batches(n)) * 128 + n * 64 for n in profile]
    in_maps = []
    for i in range(NCORES):
        gin = np.zeros((128, sum(gws)), BF16_NP)
        xo = np.zeros((128, NGRP * 2 * BG), BF16_NP)
        goff = 0
        for g in range(NGRP):
            gg_orig = i * NGRP + perm[i, g]
            sel = list(sels[gg_orig])
            nblk = profile[g]
            ctoff = goff + len(_batches(nblk)) * 128
            # granule pairs -> 128-point blocks, dummy-padded
            while len(sel) < 2 * nblk:
                sel.append(NGRAN)
            s = 0
            for bi, m in enumerate(_batches(nblk)):
                for k in range(m):
                    ua, ub = sel[2 * (s + k)], sel[2 * (s + k) + 1]
                    rb = 32 * ((s + k) % 4)
                    col = goff + bi * 128
                    gin[rb:rb + 15, col:col + 64] = pa15[:, ua * 64:
                                                         (ua + 1) * 64]
                    gin[rb:rb + 15, col + 64:col + 128] = \
                        pa15[:, ub * 64:(ub + 1) * 64]
                    gin[0:64, ctoff + (s + k) * 64:ctoff + (s + k + 1) * 64] \
                        = ct64[ua]
                    gin[64:128, ctoff + (s + k) * 64:
                        ctoff + (s + k + 1) * 64] = ct64[ub]
                s += m
            cols = slice((i * NGRP + g) * BG, (i * NGRP + g + 1) * BG)
            xo[:, g * 2 * BG:g * 2 * BG + BG] = xa4[:, cols]
            xo[:, g * 2 * BG + BG:(g + 1) * 2 * BG] = o3full[:, cols]
            goff += gws[g]
        in_maps.append({"gin": gin, "xo": xo, "bn128": bn})
    return in_maps, order, profile


def kernel(t, x, grid_points, grid_adjoints, t_edges, grid_counts,
           trace=False, tmpdir=None):
    in_maps, order, profile = _host_prep(
        t, x, grid_points, grid_adjoints, t_edges, grid_counts)
    key = ("nc", profile)
    if key not in _CACHE:
        _CACHE[key] = _build_nc(profile)
    nc = _CACHE[key]
    res = run_bass_kernel_spmd(nc, in_maps, core_ids=list(range(NCORES)),
                               trace=trace, tmpdir=tmpdir)
    _CACHE["last_result"] = res
    out_sorted = np.empty((B, 2), np.float32)
    for i in range(NCORES):
        raw = res.results[i]["o"].astype(np.float32)   # (NGRP, 3, BG)
        num = raw[:, 0:2, :]
        den = raw[:, 2, :] + EPS
        seg = (num / den[:, None, :]).transpose(0, 2, 1).reshape(BC, 2)
        out_sorted[i * BC:(i + 1) * BC] = seg
    out = np.empty((B, 2), np.float32)
    out[order] = out_sorted
    return out


# revision 28
# speedup vs baseline: 1.0744x; 1.0744x over previous
"""Bass/Trainium2 kernel for nn_KernelAMController (retrieval_knn).

Math: out(b,:) = -sum_g w_eff(b,g)*adj[tb(b),g,:] / (sum_g w_eff(b,g) + eps)
with w_eff(b,g) = exp(-2*||x_b - p_g||^2) * (counts[tb(b),g] > 0).

Strategy: the Gaussian (bandwidth 0.5) support radius is ~1.6, so after the
host sorts queries by x-coordinate each 512-sample group only interacts
with a narrow x-band of the grid. The grid is gathered host-side at
64-point granularity into per-group operand blocks of 128 points; a static
per-group-index slot profile (computed from the data, groups reordered
within each core to fit it) keeps the compiled program identical on all
cores while padding as little dummy work as possible.

Per group:
  mm1: W^T = exp(Pa^T @ Xa) over the selected blocks, K=15 split-bf16
       augmented matmul, 3 blocks CONCURRENT in the PE via row tiling
       (partition bases 0/32/64/96), exp in N=512*m ACTIVATE batches.
  mm2: Y^T(m,b) += Ct(g,m)*W^T(g,b); even slots -> PSUM 0:64, odd ->
       64:128 (col tiling, concurrent).
  Host-precomputed one-hot bin mask (o3full) * Y^T, reduced over bins by
  a +/-1 block matmul (output negation folded in). The [3,BG] result
  (num_x, num_y, den) goes back to the host, which divides/unsorts.
Inputs stream in per-group tiles spread across four engine DMA queues so
group 0's operands land ~100KB into the transfer instead of 3MB.
"""
import numpy as np
import ml_dtypes

import concourse.bass as bass
import concourse.tile as tile
from concourse import mybir, bacc
from concourse.bass_utils import run_bass_kernel_spmd

F32 = mybir.dt.float32
BF16 = mybir.dt.bfloat16
BF16_NP = ml_dtypes.bfloat16

B = 32768
G = 2500
GP = 2560          # padded grid (20 chunks of 128)
NGRAN = 40         # 64-point granules
NBINS = 20
NCORES = 8
BC = B // NCORES   # 4096 samples per core
NGRP = 8           # groups per core
BG = BC // NGRP    # 512 samples per group
MAXBLK = 6         # hard cap on 128-point blocks per group
RADIUS = 1.6       # x-distance truncation for granule selection
EPS = 1e-10

_CACHE = {}


def _batches(nblk):
    """Split a block count into mm1 batch sizes (3s then a 2/remainder)."""
    out = []
    while nblk >= 3:
        if nblk == 4:
            out += [2, 2]
            nblk = 0
            break
        out.append(3)
        nblk -= 3
    if nblk:
        out.append(nblk)
    return out


def _build_nc(profile):
    nblks = list(profile)
    tot_slots = sum(nblks)
    tot_batches = sum(len(_batches(n)) for n in nblks)

    nc = bacc.Bacc("TRN2", target_bir_lowering=False)
    # per-group widths of the grid-side gather buffer (pa batches + ct)
    gws = [len(_batches(n)) * 128 + n * 64 for n in nblks]
    gin_d = nc.dram_tensor("gin", [128, sum(gws)], BF16,
                           kind="ExternalInput")
    xo_d = nc.dram_tensor("xo", [128, NGRP * 2 * BG], BF16,
                          kind="ExternalInput")
    bn_d = nc.dram_tensor("bn128", [128, 3], BF16, kind="ExternalInput")
    o_d = nc.dram_tensor("o", [NGRP, 3, BG], F32, kind="ExternalOutput")

    with tile.TileContext(nc) as tc:
        with (
            tc.tile_pool(name="consts", bufs=1) as consts,
            tc.tile_pool(name="wt", bufs=4) as wtp,
            tc.tile_pool(name="r3", bufs=2) as r3p,
            tc.tile_pool(name="os", bufs=2) as osp,
            tc.tile_pool(name="pw", bufs=2, space="PSUM") as pwp,
            tc.tile_pool(name="py", bufs=1, space="PSUM") as pyp,
            tc.tile_pool(name="pr", bufs=1, space="PSUM") as prp,
        ):
            # warm the exp table load under the input DMAs
            dm = consts.tile([1, 1], F32)
            nc.vector.memset(dm[:], 0.0)
            dm2 = consts.tile([1, 1], F32)
            nc.scalar.activation(dm2[:], dm[:],
                                 mybir.ActivationFunctionType.Exp)

            bn_sb = consts.tile([128, 3], BF16)
            nc.gpsimd.dma_start(out=bn_sb[:], in_=bn_d[:])

            # Input tiles on two queues, never ScalarE's (exp must not sit
            # behind DMA issues in its FIFO). Engine waits coarsen to "all
            # prior DMAs on that ring", so the sync ring carries ONLY
            # group 0's two inputs — the first matmul gates on ~0.4MB
            # instead of the whole 3MB transfer.
            gin_t, xo_t = [], []
            goff = 0
            for g in range(NGRP):
                q = nc.sync if g == 0 else nc.gpsimd
                gin = consts.tile([128, gws[g]], BF16, tag=f"gin{g}")
                q.dma_start(out=gin[:], in_=gin_d[:, goff:goff + gws[g]])
                xo = consts.tile([128, 2 * BG], BF16, tag=f"xo{g}")
                q.dma_start(
                    out=xo[:], in_=xo_d[:, g * 2 * BG:(g + 1) * 2 * BG])
                gin_t.append(gin)
                xo_t.append(xo)
                goff += gws[g]

            # All profile entries are clamped to >=4 blocks so every group
            # has exactly two mm1 batches. Emission interleaves one group
            # ahead at BATCH granularity:
            #   M1E(0,0) M1E(0,1) | MM2(g,0) M1E(g+1,0) MM2(g,1) TAIL(g)
            #   M1E(g+1,1) | ... so exp(g+1,0) runs while the PE works on
            # group g's tail instead of idling ~1.8us per group.
            def m1e(g, bi, first_slot, m):
                pw = pwp.tile([128, 3, BG], F32)
                for k in range(m):
                    rb = 32 * ((first_slot + k) % 4)
                    nc.tensor.matmul(
                        pw[:, k, :],
                        lhsT=gin_t[g][rb:rb + 15, bi * 128:(bi + 1) * 128],
                        rhs=xo_t[g][rb:rb + 15, 0:BG],
                        start=True, stop=True, tile_position=(rb, 0))
                wt = wtp.tile([128, 3, BG], BF16)
                nc.scalar.activation(wt[:, 0:m, :], pw[:, 0:m, :],
                                     mybir.ActivationFunctionType.Exp)
                return wt

            def mm2_batch(g, py, wt_, ps, pm):
                nblk = nblks[g]
                ctoff = len(_batches(nblk)) * 128
                for k in range(pm):
                    sl = ps + k
                    out = py[0:64] if sl % 2 == 0 else py[64:128]
                    nc.tensor.matmul(
                        out,
                        lhsT=gin_t[g][:, ctoff + sl * 64:
                                      ctoff + (sl + 1) * 64],
                        rhs=wt_[:, k, :], start=(sl < 2),
                        stop=(sl >= nblk - 2), skip_group_check=True)

            def tail(g, py):
                # bin-select then reduce over bins (negation in bn128);
                # rows 60:64 / 124:128 of o3full are host-zeroed and the
                # matching py rows are exact zeros (ct pad columns)
                r3 = r3p.tile([128, BG], BF16)
                nc.vector.tensor_mul(r3[:], py[:], xo_t[g][:, BG:2 * BG])
                pr = prp.tile([3, BG], F32)
                nc.tensor.matmul(pr[:], lhsT=bn_sb[:], rhs=r3[:],
                                 start=True, stop=True)
                osb = osp.tile([3, BG], F32)
                nc.vector.tensor_copy(osb[:], pr[:])
                nc.sync.dma_start(out=o_d[g], in_=osb[:])

            bat = [_batches(n) for n in nblks]
            slot0 = [[sum(bs[:i]) for i in range(len(bs))] for bs in bat]
            wts = {}
            for bi in range(2):
                wts[(0, bi)] = m1e(0, bi, slot0[0][bi], bat[0][bi])
            for g in range(NGRP):
                py = pyp.tile([128, BG], F32)
                mm2_batch(g, py, wts.pop((g, 0)), slot0[g][0], bat[g][0])
                mm2_batch(g, py, wts.pop((g, 1)), slot0[g][1], bat[g][1])
                if g + 1 < NGRP:
                    wts[(g + 1, 0)] = m1e(g + 1, 0, slot0[g + 1][0],
                                          bat[g + 1][0])
                tail(g, py)
                if g + 1 < NGRP:
                    wts[(g + 1, 1)] = m1e(g + 1, 1, slot0[g + 1][1],
                                          bat[g + 1][1])
    nc.compile()
    return nc


def _host_prep(t, x, grid_points, grid_adjoints, t_edges, grid_counts):
    t = np.asarray(t, np.float32).reshape(B)
    x = np.asarray(x, np.float32)
    gp = np.asarray(grid_points, np.float32)
    adj = np.asarray(grid_adjoints, np.float32)
    te = np.asarray(t_edges, np.float32)
    cnt = np.asarray(grid_counts)

    order = np.argsort(x[:, 0], kind="stable")

    # granule x-extents (points are x-major: idx = ix*50 + iy)
    gx = gp[:, 0]
    gran_xmin = np.array([gx[64 * u] for u in range(NGRAN)], np.float32)
    gran_xmax = np.array([gx[min(64 * u + 63, G - 1)]
                          for u in range(NGRAN)], np.float32)

    # grid operands, bf16 hi/lo split; granule NGRAN is an all-dummy pad
    # (exponent -1e30 -> w=0, adjoints 0)
    p5 = np.zeros((5, GP + 64), np.float32)
    p5[0, :G] = 4.0 * gp[:, 0]
    p5[1, :G] = 4.0 * gp[:, 1]
    p5[2, :G] = -2.0
    p5[3, :G] = -2.0
    p5[4, :G] = -2.0 * (gp[:, 0] ** 2 + gp[:, 1] ** 2)
    p5[4, G:] = -1e30
    ph = p5.astype(BF16_NP)
    pl = (p5 - ph.astype(np.float32)).astype(BF16_NP)
    pa15 = np.concatenate([ph, ph, pl], axis=0)        # (15, GP+64) bf16

    mask = (cnt > 0).astype(np.float32)                # (20, G)
    ct = np.zeros((GP + 64, 64), np.float32)
    ct[:G, 0:20] = (mask * adj[:, :, 0]).T
    ct[:G, 20:40] = (mask * adj[:, :, 1]).T
    ct[:G, 40:60] = mask.T
    ct64 = ct.reshape(NGRAN + 1, 64, 64).astype(BF16_NP)

    # per-group granule selection on the x-sorted data
    xs0 = x[order, 0]
    sels, nblks = [], []
    for gg in range(B // BG):
        seg = xs0[gg * BG:(gg + 1) * BG]
        a, b = seg.min(), seg.max()
        d = np.maximum(np.maximum(gran_xmin - b, a - gran_xmax), 0.0)
        near = np.argsort(d, kind="stable")[:2 * MAXBLK]
        sel = np.sort(near[d[near] <= RADIUS])
        sels.append(sel)
        nblks.append((len(sel) + 1) // 2)

    # reorder groups within each core (descending block count) and derive
    # the static per-index profile shared by all cores
    nblks = np.array(nblks).reshape(NCORES, NGRP)
    perm = np.argsort(-nblks, axis=1, kind="stable")   # (NCORES, NGRP)
    profile = tuple(max(int(v), 4) for v in
                    np.max(np.take_along_axis(nblks, perm, axis=1), axis=0))
    order = order.reshape(NCORES, NGRP, BG)
    order = np.take_along_axis(order, perm[:, :, None], axis=1).reshape(-1)

    ts = t[order]
    xs = x[order]

    # x augmentation rows: [xh(2), sqh(2), 1, xl(2), sql(2), 0, xh(2),
    # sqh(2), 1] matching pa15 = [ph, ph, pl]; replicated at partition
    # bases 0/32/64/96 for PE row tiling
    xT = xs.T
    sq = xT * xT
    xh = xT.astype(BF16_NP)
    xl = (xT - xh.astype(np.float32)).astype(BF16_NP)
    sqh = sq.astype(BF16_NP)
    sql = (sq - sqh.astype(np.float32)).astype(BF16_NP)
    xaug = np.zeros((15, B), BF16_NP)
    for base in (0, 5, 10):
        xaug[base + 0:base + 2] = xl if base == 5 else xh
        xaug[base + 2:base + 4] = sql if base == 5 else sqh
        xaug[base + 4] = BF16_NP(0.0 if base == 5 else 1.0)
    xa4 = np.zeros((128, B), BF16_NP)
    for rb in (0, 32, 64, 96):
        xa4[rb:rb + 15] = xaug

    # host-computed one-hot bin mask, d-major layout matching py rows
    tb = np.clip(np.searchsorted(te[1:NBINS], ts, side="left"),
                 0, NBINS - 1)
    oh = np.zeros((NBINS, B), BF16_NP)
    oh[tb, np.arange(B)] = BF16_NP(1.0)
    o3full = np.zeros((128, B), BF16_NP)
    for half in (0, 64):
        for dd in range(3):
            o3full[half + dd * 20:half + (dd + 1) * 20] = oh

    bn = np.zeros((128, 3), np.float32)
    for dd in range(3):
        v = 1.0 if dd == 2 else -1.0
        bn[dd * 20:(dd + 1) * 20, dd] = v
        bn[64 + dd * 20:64 + (dd + 1) * 20, dd] = v
    bn = bn.astype(BF16_NP)

    # per-group widths of the grid-side gather buffer (pa batches + ct)
    gws = [len(_batches(n)) * 128 + n * 64 for n in profile]
    in_maps = []
    for i in range(NCORES):
        gin = np.zeros((128, sum(gws)), BF16_NP)
        xo = np.zeros((128, NGRP * 2 * BG), BF16_NP)
        goff = 0
        for g in range(NGRP):
            gg_orig = i * NGRP + perm[i, g]
            sel = list(sels[gg_orig])
            nblk = profile[g]
            ctoff = goff + len(_batches(nblk)) * 128
            # granule pairs -> 128-point blocks, dummy-padded
            while len(sel) < 2 * nblk:
                sel.append(NGRAN)
            s = 0
            for bi, m in enumerate(_batches(nblk)):
                for k in range(m):
                    ua, ub = sel[2 * (s + k)], sel[2 * (s + k) + 1]
                    rb = 32 * ((s + k) % 4)
                    col = goff + bi * 128
                    gin[rb:rb + 15, col:col + 64] = pa15[:, ua * 64:
                                                         (ua + 1) * 64]
                    gin[rb:rb + 15, col + 64:col + 128] = \
                        pa15[:, ub * 64:(ub + 1) * 64]
                    gin[0:64, ctoff + (s + k) * 64:ctoff + (s + k + 1) * 64] \
                        = ct64[ua]
                    gin[64:128, ctoff + (s + k) * 64:
                        ctoff + (s + k + 1) * 64] = ct64[ub]
                s += m
            cols = slice((i * NGRP + g) * BG, (i * NGRP + g + 1) * BG)
            xo[:, g * 2 * BG:g * 2 * BG + BG] = xa4[:, cols]
            xo[:, g * 2 * BG + BG:(g + 1) * 2 * BG] = o3full[:, cols]
            goff += gws[g]
        in_maps.append({"gin": gin, "xo": xo, "bn128": bn})
    return in_maps, order, profile


def kernel(t, x, grid_points, grid_adjoints, t_edges, grid_counts,
           trace=False, tmpdir=None):
    in_maps, order, profile = _host_prep(
        t, x, grid_points, grid_adjoints, t_edges, grid_counts)
    key = ("nc", profile)
    if key not in _CACHE:
        _CACHE[key] = _build_nc(profile)
    nc = _CACHE[key]
    res = run_bass_kernel_spmd(nc, in_maps, core_ids=list(range(NCORES)),
                               trace=trace, tmpdir=tmpdir)
    _CACHE["last_result"] = res
    out_sorted = np.empty((B, 2), np.float32)
    for i in range(NCORES):
        raw = res.results[i]["o"].astype(np.float32)   # (NGRP, 3, BG)
        num = raw[:, 0:2, :]
        den = raw[:, 2, :] + EPS
        seg = (num / den[:, None, :]).transpose(0, 2, 1).reshape(BC, 2)
        out_sorted[i * BC:(i + 1) * BC] = seg
    out = np.empty((B, 2), np.float32)
    out[order] = out_sorted
    return out
